# revision 19
# baseline (speedup 1.0000x reference)
"""K-means nearest-centroid assignment on Trainium2, data-parallel across 8 cores.

Reference computes argmin_k ||x_n - c_k||^2 == argmax_k (x_n . c_k - 0.5*||c_k||^2).
Each core gets N/8 points (transposed on host so the contraction dim C lands on
SBUF partitions) and the full centroid table.

Per 128-point subtile, K is split into two 1024-wide halves with a
[128, 1024] PSUM tile each (2 tags x 2 bufs = all 8 banks):
  PE  : subtiles run in PAIRS. First 8 contraction-2 bf16 hi/lo bias
        matmuls (-0.5||c||^2) open all 4 PSUM tiles' accumulation groups
        (start=True): K-tile j sits in 32-row group j via
        tile_position=(32j, 0) so the 4 groups stream concurrently, and the
        two subtiles of a pair are adjacent so each ones weight-load is
        shared. Then per subtile 16 fp32r matmuls accumulate x.cT
        (c-outer / K-tile-inner: each x-chunk [128,128] is loaded once and
        streams all 4 K-tiles across both halves => 4 weight loads).
  DVE : per half Max8 -> top-8 values (m8A, m8B). MaxIndex searches half A
        for m8A[0]: always valid within A, first-occurrence tie order.
  ACT : z = Sign(m8B[0] - ps_B) on half B (0 at B's argmax, +1 elsewhere),
  Pool: w = z * iota_B,
  ACT : copy-accumulate w -> s_B = SUM_B - argmax_B.
  Pool: tiny uint32->fp32 converting copy of the MaxIndex result.
  Out : per point (m_A, k_A, m_B, s_B); host picks the winning half
        (>= keeps first-occurrence order): idx = m_A >= m_B ? k_A
        : SUM_B - s_B (all integer sums exact in fp32, < 2^24).
"""

import sys

sys.path.insert(0, "/opt/trn_rl_repo")

import numpy as np

import concourse.bass as bass
import concourse.bacc as bacc
import concourse.mybir as mybir
from concourse.tile import TileContext

N, C, K = 131072, 512, 2048
NCORES = 8
P = 128
KT = 512              # psum bank width in fp32 / fp32 matmul max moving dim
NKT = K // KT         # 4 K-tiles
NCC = C // P          # 4 contraction chunks
ST = 512              # points per supertile (xT DMA free dim)
KH = K // 2           # half-K split between the DVE and ACT/Pool extractors
SUM_B = (K - 1) * K // 2 - (KH - 1) * KH // 2   # sum of iota over half B
SREC = 18             # per-subtile stage record: m8_A[8], k_A, m8_B[8], s_B

F32 = mybir.dt.float32
F32R = mybir.dt.float32r
BF16 = mybir.dt.bfloat16
U32 = mybir.dt.uint32
MM_DT = F32R


def build_nc(nloc, mm_dt=MM_DT, reps=1):
    """One SPMD program: nloc points per core, full K centroids.

    reps > 1 wraps the whole pass in a hardware loop (identical work each
    trip) so wall-clock benches get a long, overhead-dominating signal;
    the graded path uses reps=1.
    """
    nsuper = nloc // ST
    nsub = ST // P

    nc = bacc.Bacc(None, target_bir_lowering=False)
    xT = nc.declare_dram_parameter("xT", [C, nloc], mm_dt, isOutput=False)
    cT = nc.declare_dram_parameter("cT", [C, K], mm_dt, isOutput=False)
    # bias2[32j] / bias2[32j+1] = bf16 hi / lo parts of -0.5*||c_k||^2,
    # replicated into each 32-row group so bias matmul j can run in row
    # group j via tile_position.
    bias2 = nc.declare_dram_parameter("bias2", [P, K], BF16, isOutput=False)
    # iota[p, k] = k (global centroid index), replicated across partitions.
    iota = nc.declare_dram_parameter("iota", [P, K], F32, isOutput=False)
    # out viewed as [nloc, 18] = per-point (m8_A[8], k_A, m8_B[8], s_B).
    out = nc.declare_dram_parameter("out", [SREC * nloc], F32, isOutput=True)

    with TileContext(nc) as tc:
        with (
            tc.tile_pool(name="const", bufs=1) as const_pool,
            tc.tile_pool(name="xin", bufs=3) as xin_pool,
            tc.tile_pool(name="big", bufs=3) as big_pool,
            tc.tile_pool(name="mx", bufs=4) as m_pool,
            tc.tile_pool(name="res", bufs=3) as res_pool,
            tc.tile_pool(name="psum", bufs=2, space="PSUM") as psum_pool,
        ):
            # DMA order follows first use: the bias matmuls open every
            # accumulation group, then chunk-0 main matmuls, etc.
            bias2_t = const_pool.tile([P, K], BF16, tag="bias2")
            nc.sync.dma_start(out=bias2_t[:], in_=bias2[:, :])
            cT_tiles = []
            for c in range(NCC):
                t = const_pool.tile([P, K], mm_dt, tag=f"cT{c}")
                nc.sync.dma_start(out=t[:], in_=cT[c * P:(c + 1) * P, :])
                cT_tiles.append(t)
            iota_t = const_pool.tile([P, K], F32, tag="iota")
            nc.sync.dma_start(out=iota_t[:], in_=iota[:, :])
            ones_t = const_pool.tile([P, P], BF16, tag="ones")
            nc.vector.memset(ones_t[:], 1.0)

            def bias_mms(tiles_ab):
                # tiles_ab: [(psA, psB), (psA, psB)] for the subtile pair.
                # K-tile j -> row group 32j; adjacent identical lhsT slices
                # let the PE reuse one ones-load per row group.
                for j in range(NKT):
                    r = 32 * j
                    half, jj = divmod(j, NKT // 2)
                    for psab in tiles_ab:
                        nc.tensor.matmul(
                            psab[half][:, jj * KT:(jj + 1) * KT],
                            lhsT=ones_t[r:r + 2, :],
                            rhs=bias2_t[r:r + 2, j * KT:(j + 1) * KT],
                            start=True,
                            stop=False,
                            tile_position=(r, 0),
                        )

            def main_mms(psab, xall, s):
                for c in range(NCC):
                    for j in range(NKT):
                        half, jj = divmod(j, NKT // 2)
                        nc.tensor.matmul(
                            psab[half][:, jj * KT:(jj + 1) * KT],
                            lhsT=xall[:, c * ST + s * P:c * ST + (s + 1) * P],
                            rhs=cT_tiles[c][:, j * KT:(j + 1) * KT],
                            start=False,
                            stop=(c == NCC - 1),
                        )

            def extract(psab, stage_t, s):
                # stage record per subtile: [0:8]=m8_A, [8]=k_A,
                # [9:17]=m8_B, [17]=s_B  (Max8 writes its top-8 block
                # straight into the staging tile - no copy ops needed).
                ps_a, ps_b = psab
                b0 = SREC * s
                nc.vector.max(stage_t[:, b0:b0 + 8], ps_a[:])
                i8 = m_pool.tile([P, 8], U32, tag="i8")
                nc.vector.max_index(i8[:], stage_t[:, b0:b0 + 8], ps_a[:])
                nc.vector.max(stage_t[:, b0 + 9:b0 + 17], ps_b[:])
                nc.gpsimd.tensor_copy(
                    stage_t[:, b0 + 8:b0 + 9], i8[:, 0:1]
                )
                z_t = big_pool.tile([P, KH], F32, tag="z")
                nc.scalar.activation(
                    out=z_t[:],
                    in_=ps_b[:],
                    func=mybir.ActivationFunctionType.Sign,
                    bias=stage_t[:, b0 + 9:b0 + 10],
                    scale=-1.0,
                )
                wb_t = big_pool.tile([P, KH], F32, tag="wb")
                nc.gpsimd.tensor_mul(wb_t[:], z_t[:], iota_t[:, KH:])
                wo_t = big_pool.tile([P, KH], F32, tag="wo")
                nc.scalar.activation(
                    out=wo_t[:],
                    in_=wb_t[:],
                    func=mybir.ActivationFunctionType.Copy,
                    accum_out=stage_t[:, b0 + 17:b0 + 18],
                )

            def body():
                for st in range(nsuper):
                    n0 = st * ST
                    # one DMA per supertile: chunk c of xT lands in cols
                    # [c*ST, (c+1)*ST) of a single [P, NCC*ST] tile.
                    xall = xin_pool.tile([P, NCC * ST], mm_dt, tag="xall")
                    nc.sync.dma_start(
                        out=xall[:],
                        in_=xT[:, n0:n0 + ST].rearrange(
                            "(c p) w -> p c w", p=P
                        ),
                    )
                    stage_t = res_pool.tile([P, SREC * nsub], F32, tag="st")
                    for sp in range(nsub // 2):
                        s0, s1 = 2 * sp, 2 * sp + 1
                        pair = []
                        for half_i in range(2):
                            ps_a = psum_pool.tile(
                                [P, KH], mybir.dt.float32,
                                tag="psA", name=f"psA{half_i}",
                            )
                            ps_b = psum_pool.tile(
                                [P, KH], mybir.dt.float32,
                                tag="psB", name=f"psB{half_i}",
                            )
                            pair.append((ps_a, ps_b))
                        bias_mms(pair)
                        main_mms(pair[0], xall, s0)
                        extract(pair[0], stage_t, s0)
                        main_mms(pair[1], xall, s1)
                        extract(pair[1], stage_t, s1)
                    nc.sync.dma_start(
                        out=out[SREC * n0:SREC * (n0 + ST)].rearrange(
                            "(s p q) -> p s q", p=P, q=SREC
                        ),
                        in_=stage_t[:],
                    )

            if reps == 1:
                body()
            else:
                with tc.For_i(0, reps):
                    body()
    nc.finalize()
    return nc


def make_in_maps(inp, centroids, nloc=None, ncores=NCORES):
    import ml_dtypes

    inp = np.asarray(inp, dtype=np.float32)
    centroids = np.asarray(centroids, dtype=np.float32)
    if nloc is None:
        nloc = inp.shape[0] // ncores
    cT = np.ascontiguousarray(centroids.T)
    c2 = np.sum(centroids.astype(np.float64) ** 2, axis=1)
    bias_row = (-0.5 * c2).astype(np.float32)
    bias_hi = bias_row.astype(ml_dtypes.bfloat16)
    bias_lo = (bias_row - bias_hi.astype(np.float32)).astype(ml_dtypes.bfloat16)
    bias2 = np.zeros((P, K), dtype=ml_dtypes.bfloat16)
    for j in range(NKT):
        bias2[32 * j] = bias_hi
        bias2[32 * j + 1] = bias_lo
    iota = np.ascontiguousarray(
        np.broadcast_to(np.arange(K, dtype=np.float32)[None, :], (P, K))
    )
    in_maps = []
    for i in range(ncores):
        xl = inp[i * nloc:(i + 1) * nloc]
        in_maps.append(
            {
                "xT": np.ascontiguousarray(xl.T),
                "cT": cT,
                "bias2": bias2,
                "iota": iota,
            }
        )
    return in_maps


def unshard_out(arr):
    """Per-core [18*nloc] fp32 (m8_A[8], k_A, m8_B[8], s_B) -> [nloc] int32."""
    v = np.asarray(arr, dtype=np.float64).reshape(-1, SREC)
    ma, ka, mb, sb = v[:, 0], v[:, 8], v[:, 9], v[:, 17]
    return np.rint(np.where(ma >= mb, ka, SUM_B - sb)).astype(np.int32)


def kernel(inp, centroids):
    from concourse.bass_utils import run_bass_kernel_spmd

    nloc = N // NCORES
    nc = build_nc(nloc)
    in_maps = make_in_maps(inp, centroids, nloc=nloc)
    res = run_bass_kernel_spmd(nc, in_maps, core_ids=list(range(NCORES)))
    parts = [unshard_out(res.results[i]["out"]) for i in range(NCORES)]
    return np.concatenate(parts)


# revision 20
# speedup vs baseline: 1.1522x; 1.1522x over previous
"""K-means nearest-centroid assignment on Trainium2, data-parallel across 8 cores.

Reference computes argmin_k ||x_n - c_k||^2 == argmax_k (x_n . c_k - 0.5*||c_k||^2).
Each core gets N/8 points (transposed on host so the contraction dim C lands on
SBUF partitions) and the full centroid table.

Per 128-point subtile, K is split into two 1024-wide halves with a
[128, 1024] PSUM tile each (2 tags x 2 bufs = all 8 banks):
  PE  : subtiles run in PAIRS. First 8 contraction-2 bf16 hi/lo bias
        matmuls (-0.5||c||^2) open all 4 PSUM tiles' accumulation groups
        (start=True): K-tile j sits in 32-row group j via
        tile_position=(32j, 0) so the 4 groups stream concurrently, and the
        two subtiles of a pair are adjacent so each ones weight-load is
        shared. Then per subtile 16 fp32r matmuls accumulate x.cT
        (c-outer / K-tile-inner: each x-chunk [128,128] is loaded once and
        streams all 4 K-tiles across both halves => 4 weight loads).
  DVE : per half Max8 -> top-8 values (m8A, m8B). MaxIndex searches half A
        for m8A[0]: always valid within A, first-occurrence tie order.
  ACT : z = Sign(m8B[0] - ps_B) on half B (0 at B's argmax, +1 elsewhere),
  Pool: w = z * iota_B,
  ACT : copy-accumulate w -> s_B = SUM_B - argmax_B.
  Pool: tiny uint32->fp32 converting copy of the MaxIndex result.
  Out : per point (m_A, k_A, m_B, s_B); host picks the winning half
        (>= keeps first-occurrence order): idx = m_A >= m_B ? k_A
        : SUM_B - s_B (all integer sums exact in fp32, < 2^24).
"""

import sys

sys.path.insert(0, "/opt/trn_rl_repo")

import numpy as np

import concourse.bass as bass
import concourse.bacc as bacc
import concourse.mybir as mybir
from concourse.tile import TileContext

N, C, K = 131072, 512, 2048
NCORES = 8
P = 128
KT = 512              # psum bank width in fp32 / fp32 matmul max moving dim
NKT = K // KT         # 4 K-tiles
NCC = C // P          # 4 contraction chunks
ST = 512              # points per supertile (xT DMA free dim)
KH = K // 2           # half-K split between the DVE and ACT/Pool extractors
SUM_B = (K - 1) * K // 2 - (KH - 1) * KH // 2   # sum of iota over half B

F32 = mybir.dt.float32
F32R = mybir.dt.float32r
BF16 = mybir.dt.bfloat16
U32 = mybir.dt.uint32
MM_DT = F32R


def build_nc(nloc, mm_dt=MM_DT, reps=1):
    """One SPMD program: nloc points per core, full K centroids.

    reps > 1 wraps the whole pass in a hardware loop (identical work each
    trip) so wall-clock benches get a long, overhead-dominating signal;
    the graded path uses reps=1.
    """
    nsuper = nloc // ST
    nsub = ST // P

    nc = bacc.Bacc(None, target_bir_lowering=False)
    xT = nc.declare_dram_parameter("xT", [C, nloc], mm_dt, isOutput=False)
    cT = nc.declare_dram_parameter("cT", [C, K], mm_dt, isOutput=False)
    # bias2[32j] / bias2[32j+1] = bf16 hi / lo parts of -0.5*||c_k||^2,
    # replicated into each 32-row group so bias matmul j can run in row
    # group j via tile_position.
    bias2 = nc.declare_dram_parameter("bias2", [P, K], BF16, isOutput=False)
    # iota[p, k] = k (global centroid index), replicated across partitions.
    iota = nc.declare_dram_parameter("iota", [P, K], F32, isOutput=False)
    # out viewed as [nloc, 4] = per-point (m_A, k_A, m_B, s_B).
    out = nc.declare_dram_parameter("out", [4 * nloc], F32, isOutput=True)

    with TileContext(nc) as tc:
        with (
            tc.tile_pool(name="const", bufs=1) as const_pool,
            tc.tile_pool(name="xin", bufs=3) as xin_pool,
            tc.tile_pool(name="big", bufs=3) as big_pool,
            tc.tile_pool(name="mx", bufs=4) as m_pool,
            tc.tile_pool(name="res", bufs=3) as res_pool,
            tc.tile_pool(name="psum", bufs=2, space="PSUM") as psum_pool,
        ):
            # DMA order follows first use: the bias matmuls open every
            # accumulation group, then chunk-0 main matmuls, etc.
            bias2_t = const_pool.tile([P, K], BF16, tag="bias2")
            nc.sync.dma_start(out=bias2_t[:], in_=bias2[:, :])
            cT_tiles = []
            for c in range(NCC):
                t = const_pool.tile([P, K], mm_dt, tag=f"cT{c}")
                nc.sync.dma_start(out=t[:], in_=cT[c * P:(c + 1) * P, :])
                cT_tiles.append(t)
            iota_t = const_pool.tile([P, K], F32, tag="iota")
            nc.sync.dma_start(out=iota_t[:], in_=iota[:, :])
            ones_t = const_pool.tile([P, P], BF16, tag="ones")
            nc.vector.memset(ones_t[:], 1.0)

            def bias_mms(tiles_ab):
                # tiles_ab: [(psA, psB), (psA, psB)] for the subtile pair.
                # K-tile j -> row group 32j; adjacent identical lhsT slices
                # let the PE reuse one ones-load per row group.
                for j in range(NKT):
                    r = 32 * j
                    half, jj = divmod(j, NKT // 2)
                    for psab in tiles_ab:
                        nc.tensor.matmul(
                            psab[half][:, jj * KT:(jj + 1) * KT],
                            lhsT=ones_t[r:r + 2, :],
                            rhs=bias2_t[r:r + 2, j * KT:(j + 1) * KT],
                            start=True,
                            stop=False,
                            tile_position=(r, 0),
                        )

            def main_mms(psab, xall, s):
                for c in range(NCC):
                    for j in range(NKT):
                        half, jj = divmod(j, NKT // 2)
                        nc.tensor.matmul(
                            psab[half][:, jj * KT:(jj + 1) * KT],
                            lhsT=xall[:, c * ST + s * P:c * ST + (s + 1) * P],
                            rhs=cT_tiles[c][:, j * KT:(j + 1) * KT],
                            start=False,
                            stop=(c == NCC - 1),
                        )

            def extract(psab, stage_t, s):
                ps_a, ps_b = psab
                m8a = m_pool.tile([P, 8], mybir.dt.float32, tag="m8a")
                nc.vector.max(m8a[:], ps_a[:])
                i8 = m_pool.tile([P, 8], U32, tag="i8")
                nc.vector.max_index(i8[:], m8a[:], ps_a[:])
                m8b = m_pool.tile([P, 8], mybir.dt.float32, tag="m8b")
                nc.vector.max(m8b[:], ps_b[:])
                nc.gpsimd.tensor_copy(
                    stage_t[:, 4 * s:4 * s + 1], m8a[:, 0:1]
                )
                nc.gpsimd.tensor_copy(
                    stage_t[:, 4 * s + 1:4 * s + 2], i8[:, 0:1]
                )
                nc.gpsimd.tensor_copy(
                    stage_t[:, 4 * s + 2:4 * s + 3], m8b[:, 0:1]
                )
                z_t = big_pool.tile([P, KH], F32, tag="z")
                nc.scalar.activation(
                    out=z_t[:],
                    in_=ps_b[:],
                    func=mybir.ActivationFunctionType.Sign,
                    bias=m8b[:, 0:1],
                    scale=-1.0,
                )
                wb_t = big_pool.tile([P, KH], F32, tag="wb")
                nc.gpsimd.tensor_mul(wb_t[:], z_t[:], iota_t[:, KH:])
                wo_t = big_pool.tile([P, KH], F32, tag="wo")
                nc.scalar.activation(
                    out=wo_t[:],
                    in_=wb_t[:],
                    func=mybir.ActivationFunctionType.Copy,
                    accum_out=stage_t[:, 4 * s + 3:4 * s + 4],
                )

            def body():
                for st in range(nsuper):
                    n0 = st * ST
                    # one DMA per supertile: chunk c of xT lands in cols
                    # [c*ST, (c+1)*ST) of a single [P, NCC*ST] tile.
                    xall = xin_pool.tile([P, NCC * ST], mm_dt, tag="xall")
                    nc.sync.dma_start(
                        out=xall[:],
                        in_=xT[:, n0:n0 + ST].rearrange(
                            "(c p) w -> p c w", p=P
                        ),
                    )
                    stage_t = res_pool.tile([P, 4 * nsub], F32, tag="st")
                    for sp in range(nsub // 2):
                        s0, s1 = 2 * sp, 2 * sp + 1
                        pair = []
                        for half_i in range(2):
                            ps_a = psum_pool.tile(
                                [P, KH], mybir.dt.float32,
                                tag="psA", name=f"psA{half_i}",
                            )
                            ps_b = psum_pool.tile(
                                [P, KH], mybir.dt.float32,
                                tag="psB", name=f"psB{half_i}",
                            )
                            pair.append((ps_a, ps_b))
                        bias_mms(pair)
                        main_mms(pair[0], xall, s0)
                        extract(pair[0], stage_t, s0)
                        main_mms(pair[1], xall, s1)
                        extract(pair[1], stage_t, s1)
                    nc.sync.dma_start(
                        out=out[4 * n0:4 * (n0 + ST)].rearrange(
                            "(s p q) -> p s q", p=P, q=4
                        ),
                        in_=stage_t[:],
                    )

            if reps == 1:
                body()
            else:
                with tc.For_i(0, reps):
                    body()
    nc.finalize()
    return nc


def make_in_maps(inp, centroids, nloc=None, ncores=NCORES):
    import ml_dtypes

    inp = np.asarray(inp, dtype=np.float32)
    centroids = np.asarray(centroids, dtype=np.float32)
    if nloc is None:
        nloc = inp.shape[0] // ncores
    cT = np.ascontiguousarray(centroids.T)
    c2 = np.sum(centroids.astype(np.float64) ** 2, axis=1)
    bias_row = (-0.5 * c2).astype(np.float32)
    bias_hi = bias_row.astype(ml_dtypes.bfloat16)
    bias_lo = (bias_row - bias_hi.astype(np.float32)).astype(ml_dtypes.bfloat16)
    bias2 = np.zeros((P, K), dtype=ml_dtypes.bfloat16)
    for j in range(NKT):
        bias2[32 * j] = bias_hi
        bias2[32 * j + 1] = bias_lo
    iota = np.ascontiguousarray(
        np.broadcast_to(np.arange(K, dtype=np.float32)[None, :], (P, K))
    )
    in_maps = []
    for i in range(ncores):
        xl = inp[i * nloc:(i + 1) * nloc]
        in_maps.append(
            {
                "xT": np.ascontiguousarray(xl.T),
                "cT": cT,
                "bias2": bias2,
                "iota": iota,
            }
        )
    return in_maps


def unshard_out(arr):
    """Per-core [4*nloc] fp32 (m_A, k_A, m_B, s_B) -> [nloc] int32."""
    v = np.asarray(arr, dtype=np.float64).reshape(-1, 4)
    ma, ka, mb, sb = v[:, 0], v[:, 1], v[:, 2], v[:, 3]
    return np.rint(np.where(ma >= mb, ka, SUM_B - sb)).astype(np.int32)


def kernel(inp, centroids):
    from concourse.bass_utils import run_bass_kernel_spmd

    nloc = N // NCORES
    nc = build_nc(nloc)
    in_maps = make_in_maps(inp, centroids, nloc=nloc)
    res = run_bass_kernel_spmd(nc, in_maps, core_ids=list(range(NCORES)))
    parts = [unshard_out(res.results[i]["out"]) for i in range(NCORES)]
    return np.concatenate(parts)
